# revision 2
# baseline (speedup 1.0000x reference)
"""MoE adapter (top-1 of 4 experts, dense all-expert reference) on 8 TRN2 NeuronCores.

Strategy (v2: fp8 DoubleRow)
----------------------------
Data-parallel over the 32768 tokens (4096 per core); expert weights replicated.

Since 4 experts x H=192 = 768, the four expert MLPs stack into two dense
768x768 matmuls.  Both big matmuls run in fp8(e4m3) with DoubleRow perf mode
(256-deep contraction per instruction, ~1.4-2x bf16 throughput):

    h    = gelu(512*(x8 @ W1_8)/512 + b1 - mask_bias)   # [768h, tok] fp8
    y64  = h @ W2_8 + one_hot @ (64 b2)                  # [tok, 768] psum
    out  = (y64 + 64 x) / 64                             # /64 on host

Key tricks vs the bf16 baseline:
  * top-1 masking is done by accumulating a -2^20 bias into the pre-gelu
    PSUM rows of non-selected experts (one tiny K=4 matmul per h-chunk);
    gelu(-2048) == 0, so no mask multiply / expand anywhere.
  * router is a single fp16 pass (half the hi/lo bf16 cost); argmax is
    transpose-free: pairwise partition max + is_equal on [4, tok].
  * scales: x8 = 8x, W*_8 = 64W -> mm1 psum = 512*(x@w1), removed by the
    gelu's scale=1/512.  mm2 psum = 64*y; skip tensor is shipped as 64x fp16
    so one DVE add drains psum+skip, and the host divides the fp16 result
    by 64 (exact exponent shift).
  * fp16 everywhere else keeps rel-err ~1e-3 << 2e-2 budget.
"""

import numpy as np
import ml_dtypes

import concourse.bass as bass
import concourse.mybir as mybir
import concourse.tile as tile
from concourse import bacc
from concourse.bass_utils import run_bass_kernel_spmd

BF16 = ml_dtypes.bfloat16
FP16 = np.float16
FP8 = ml_dtypes.float8_e4m3
F32 = np.float32

B, S, D = 16, 2048, 768
H, E = 192, 4
N_CORES = 8
TOK_TOTAL = B * S                 # 32768
TOK = TOK_TOTAL // N_CORES        # 4096 tokens per core
TILE = 512                        # tokens per pipeline tile
N_TILES = TOK // TILE             # 8
SUBT = TILE // 128                # 4 token subtiles of 128
KC = D // 128                     # 6 contraction chunks
DC = KC // 2                      # 3 DoubleRow double-chunks

XS = 8.0                          # x fp8 scale
WS = 64.0                         # weight fp8 scale
OS = 64.0                         # output / skip scale
MBIG = float(2 ** 20)             # pre-gelu mask offset (=> -2048 post scale)

_NC_CACHE = None


def _build_bass():
    dt = mybir.dt
    DR = mybir.MatmulPerfMode.DoubleRow
    nc = bacc.Bacc("TRN2", target_bir_lowering=False)

    xf16 = nc.dram_tensor("xf16", [D, TOK], dt.float16, kind="ExternalInput")
    x8 = nc.dram_tensor("x8", [D, TOK], dt.float8e4, kind="ExternalInput")
    xs16 = nc.dram_tensor("xs16", [TOK, D], dt.float16, kind="ExternalInput")
    w18 = nc.dram_tensor("w18", [D, D], dt.float8e4, kind="ExternalInput")
    w28 = nc.dram_tensor("w28", [D, D], dt.float8e4, kind="ExternalInput")
    rw16 = nc.dram_tensor("rw16", [D, E], dt.float16, kind="ExternalInput")
    rbt = nc.dram_tensor("rbt", [E, 1], dt.float32, kind="ExternalInput")
    b1r = nc.dram_tensor("b1r", [128, KC], dt.float32, kind="ExternalInput")
    een = nc.dram_tensor("een", [E, 128], dt.bfloat16, kind="ExternalInput")
    b2s = nc.dram_tensor("b2s", [E, D], dt.bfloat16, kind="ExternalInput")
    out = nc.dram_tensor("out", [TOK, D], dt.float16, kind="ExternalOutput")

    xf_r = xf16.rearrange("(c p) t -> p c t", p=128)
    x8_r = x8.rearrange("(c i p) t -> p c i t", i=2, p=128)
    w1_r = w18.rearrange("(c i p) m -> p c i m", i=2, p=128)
    w2_r = w28.rearrange("(c i p) m -> p c i m", i=2, p=128)
    rw_r = rw16.rearrange("(c p) e -> p c e", p=128)
    xs_r = xs16.rearrange("(n p) d -> p n d", p=128)
    out_r = out.rearrange("(n p) d -> p n d", p=128)

    add = mybir.AluOpType.add
    amax = mybir.AluOpType.max
    iseq = mybir.AluOpType.is_equal
    gelu = mybir.ActivationFunctionType.Gelu

    with tile.TileContext(nc) as tc:
        with (
            tc.tile_pool(name="const", bufs=1) as const,
            tc.tile_pool(name="xin", bufs=3) as xin,
            tc.tile_pool(name="hbuf", bufs=2) as hbuf,
            tc.tile_pool(name="obuf", bufs=6) as obuf,
            tc.tile_pool(name="small", bufs=3) as small,
            tc.tile_pool(name="ps_rt", bufs=2, space="PSUM") as ps_rt,
            tc.tile_pool(name="ps_h", bufs=2, space="PSUM") as ps_h,
            tc.tile_pool(name="ps_y5", bufs=2, space="PSUM") as ps_y5,
            tc.tile_pool(name="ps_y2", bufs=2, space="PSUM") as ps_y2,
        ):
            # small constants ride the gpsimd SWDGE queue; the two weight
            # matrices go on the sync HWDGE FIFO interleaved with tile-0 loads.
            rwsb = const.tile([128, KC, E], dt.float16)
            nc.gpsimd.dma_start(rwsb, rw_r)
            rbsb = const.tile([E, 1], dt.float32)
            nc.gpsimd.dma_start(rbsb, rbt[:])
            b1sb = const.tile([128, KC], dt.float32)
            nc.gpsimd.dma_start(b1sb, b1r[:])
            eesb = const.tile([E, 128], dt.bfloat16)
            nc.gpsimd.dma_start(eesb, een[:])
            b2sb = const.tile([E, D], dt.bfloat16)
            nc.gpsimd.dma_start(b2sb, b2s[:])
            w1sb = const.tile([128, DC, 2, D], dt.float8e4)
            w2sb = const.tile([128, DC, 2, D], dt.float8e4)

            def load_tiles(it):
                t0 = it * TILE
                xf = xin.tile([128, KC, TILE], dt.float16, tag="xf")
                nc.sync.dma_start(xf, xf_r[:, :, t0 : t0 + TILE])
                x8t = xin.tile([128, DC, 2, TILE], dt.float8e4, tag="x8t")
                nc.sync.dma_start(x8t, x8_r[:, :, :, t0 : t0 + TILE])
                if it == 0:
                    nc.sync.dma_start(w1sb, w1_r)
                xst = xin.tile([128, SUBT, D], dt.float16, tag="xst")
                nc.scalar.dma_start(xst, xs_r[:, it * SUBT : (it + 1) * SUBT, :])
                if it == 0:
                    nc.sync.dma_start(w2sb, w2_r)
                return xf, x8t, xst

            def router_pe(xf):
                """fp16 logits^T for one tile -> psum [4, TILE]."""
                psrt = ps_rt.tile([E, TILE], dt.float32, tag="psrt")
                for kc in range(KC):
                    nc.tensor.matmul(
                        psrt, rwsb[:, kc, :], xf[:, kc, :],
                        start=(kc == 0), stop=(kc == KC - 1),
                    )
                return psrt

            def router_mask(psrt):
                """one-hot top-1 mask mt [4, TILE] bf16 (transpose-free)."""
                lsb = small.tile([E, TILE], dt.float32, tag="lsb")
                nc.vector.tensor_scalar_add(lsb, psrt, rbsb[:, 0:1])
                mx2 = small.tile([2, TILE], dt.float32, tag="mx2")
                nc.vector.tensor_tensor(mx2, lsb[0:2], lsb[2:4], amax)
                mx1 = small.tile([1, TILE], dt.float32, tag="mx1")
                nc.vector.tensor_tensor(mx1, mx2[0:1], mx2[1:2], amax)
                mt = small.tile([E, TILE], dt.bfloat16, tag="mt")
                nc.vector.tensor_tensor(
                    mt, lsb, mx1.to_broadcast((E, TILE)), iseq
                )
                return mt

            # PE warm-up burst: spin the HAM up during the DMA head
            dummy = const.tile([128, TILE], dt.bfloat16)
            nc.vector.memset(dummy, 0.0)
            psd = ps_h.tile([128, TILE], dt.float32, tag="psh")
            for _ in range(10):
                nc.tensor.matmul(psd, dummy[:, 0:128], dummy, start=True, stop=True)

            tiles = {0: load_tiles(0)}
            mt = router_mask(router_pe(tiles[0][0]))
            tiles[1] = load_tiles(1)

            for it in range(N_TILES):
                xf, x8t, xst = tiles[it]
                if it + 2 < N_TILES:
                    tiles[it + 2] = load_tiles(it + 2)

                # ---- mm1 (fp8 DoubleRow) + mask-bias + gelu -> mh fp8 ----
                mh = hbuf.tile([128, KC, TILE], dt.float8e4, tag="mh")
                for hc in range(KC):
                    psh = ps_h.tile([128, TILE], dt.float32, tag="psh")
                    # -2^20 into non-selected experts' rows (gelu() -> 0)
                    nc.tensor.matmul(psh, eesb, mt, start=True, stop=False)
                    for dc in range(DC):
                        nc.tensor.matmul(
                            psh,
                            w1sb[:, dc, :, hc * 128 : (hc + 1) * 128],
                            x8t[:, dc, :, :],
                            start=False, stop=(dc == DC - 1),
                            perf_mode=DR,
                        )
                    nc.scalar.activation(
                        mh[:, hc, :], psh, gelu,
                        bias=b1sb[:, hc : hc + 1], scale=1.0 / (XS * WS),
                    )

                # router for tile n+1 hides its DVE chain under mm2
                psrt_n = router_pe(tiles[it + 1][0]) if it + 1 < N_TILES else None

                # ---- mm2 (fp8 DoubleRow) + b2 + skip-add, token-major ----
                for a in range(SUBT):
                    osb = obuf.tile([128, D], dt.float16, tag="osb")
                    psy5 = ps_y5.tile([128, 512], dt.float32, tag="psy5")
                    psy2 = ps_y2.tile([128, 256], dt.float32, tag="psy2")
                    mta = mt[:, a * 128 : (a + 1) * 128]
                    nc.tensor.matmul(psy5, mta, b2sb[:, 0:512], start=True, stop=False)
                    nc.tensor.matmul(psy2, mta, b2sb[:, 512:768], start=True, stop=False)
                    for dc in range(DC):
                        mha = mh[:, 2 * dc : 2 * dc + 2, a * 128 : (a + 1) * 128]
                        nc.tensor.matmul(
                            psy5, mha, w2sb[:, dc, :, 0:512],
                            start=False, stop=(dc == DC - 1), perf_mode=DR,
                        )
                        nc.tensor.matmul(
                            psy2, mha, w2sb[:, dc, :, 512:768],
                            start=False, stop=(dc == DC - 1), perf_mode=DR,
                        )
                    nc.vector.tensor_tensor(osb[:, 0:512], psy5, xst[:, a, 0:512], add)
                    nc.vector.tensor_tensor(osb[:, 512:768], psy2, xst[:, a, 512:768], add)
                    nc.scalar.dma_start(out_r[:, it * SUBT + a, :], osb)

                if psrt_n is not None:
                    mt = router_mask(psrt_n)
                del tiles[it]

    nc.compile()
    return nc


def _prep_inputs(x, router_w, router_b, w1, b1, w2, b2):
    """Host-side packing: cast/scale/transpose; returns per-core input dicts."""
    xf = np.ascontiguousarray(np.asarray(x, dtype=F32).reshape(TOK_TOTAL, D))

    rw = np.asarray(router_w, dtype=F32).astype(FP16)         # [D, E]
    rb = np.asarray(router_b, dtype=F32).reshape(E, 1)

    w1f = np.asarray(w1, dtype=F32)                           # [E, D, H]
    w2f = np.asarray(w2, dtype=F32)                           # [E, H, D]
    b1f = np.asarray(b1, dtype=F32)                           # [E, H]
    b2f = np.asarray(b2, dtype=F32)                           # [E, D]

    # experts interleaved along the stacked hidden dim (unit j of expert e at
    # index 4j + e) so the mask pattern repeats identically per 128-row chunk.
    w1s = np.ascontiguousarray(w1f.transpose(1, 2, 0).reshape(D, H * E))
    w2s = np.ascontiguousarray(w2f.transpose(1, 0, 2).reshape(H * E, D))
    w18 = (WS * w1s).astype(FP8)
    w28 = (WS * w2s).astype(FP8)
    b1all = np.ascontiguousarray(b1f.T.reshape(E * H))
    b1r = np.ascontiguousarray(b1all.reshape(KC, 128).T).astype(F32)
    b2sb = (OS * b2f).astype(BF16)

    een = np.full((E, 128), -MBIG, dtype=BF16)
    for e in range(E):
        een[e, e::E] = 0.0

    in_maps = []
    for c in range(N_CORES):
        sl = slice(c * TOK, (c + 1) * TOK)
        xc = xf[sl]
        xT = np.ascontiguousarray(xc.T)
        in_maps.append(
            {
                "xf16": xT.astype(FP16),
                "x8": (XS * xT).astype(FP8),
                "xs16": (OS * xc).astype(FP16),
                "w18": w18,
                "w28": w28,
                "rw16": rw,
                "rbt": rb,
                "b1r": b1r,
                "een": een,
                "b2s": b2sb,
            }
        )
    return in_maps


def _get_nc():
    global _NC_CACHE
    if _NC_CACHE is None:
        _NC_CACHE = _build_bass()
    return _NC_CACHE


def kernel(x, router_w, router_b, w1, b1, w2, b2, _trace=False, _trace_kwargs=None):
    in_maps = _prep_inputs(x, router_w, router_b, w1, b1, w2, b2)
    nc = _get_nc()
    res = run_bass_kernel_spmd(
        nc,
        in_maps,
        core_ids=list(range(N_CORES)),
        trace=_trace,
        **(_trace_kwargs or {}),
    )
    outs = [np.asarray(r["out"], dtype=F32) * (1.0 / OS) for r in res.results]
    full = np.concatenate(outs, axis=0).reshape(B, S, D)
    if _trace:
        kernel.last_results = res
    return full


# revision 6
# speedup vs baseline: 1.2873x; 1.2873x over previous
"""MoE adapter (top-1 of 4 experts, dense all-expert reference) on 8 TRN2 NeuronCores.

Strategy (v2: fp8 DoubleRow)
----------------------------
Data-parallel over the 32768 tokens (4096 per core); expert weights replicated.

Since 4 experts x H=192 = 768, the four expert MLPs stack into two dense
768x768 matmuls.  Both big matmuls run in fp8(e4m3) with DoubleRow perf mode
(256-deep contraction per instruction, ~1.4-2x bf16 throughput):

    h    = gelu(512*(x8 @ W1_8)/512 + b1 - mask_bias)   # [768h, tok] fp8
    y64  = h @ W2_8 + one_hot @ (64 b2)                  # [tok, 768] psum
    out  = (y64 + 64 x) / 64                             # /64 on host

Key tricks vs the bf16 baseline:
  * top-1 masking is done by accumulating a -2^20 bias into the pre-gelu
    PSUM rows of non-selected experts (one tiny K=4 matmul per h-chunk);
    gelu(-2048) == 0, so no mask multiply / expand anywhere.
  * router is a single fp16 pass (half the hi/lo bf16 cost); argmax is
    transpose-free: pairwise partition max + is_equal on [4, tok].
  * scales: x8 = 8x, W*_8 = 64W -> mm1 psum = 512*(x@w1), removed by the
    gelu's scale=1/512.  mm2 psum = 64*y; skip tensor is shipped as 64x fp16
    so one DVE add drains psum+skip, and the host divides the fp16 result
    by 64 (exact exponent shift).
  * fp16 everywhere else keeps rel-err ~1e-3 << 2e-2 budget.
"""

import numpy as np
import ml_dtypes

import concourse.bass as bass
import concourse.mybir as mybir
import concourse.tile as tile
from concourse import bacc
from concourse.bass_utils import run_bass_kernel_spmd

BF16 = ml_dtypes.bfloat16
FP16 = np.float16
FP8 = ml_dtypes.float8_e4m3
F32 = np.float32

B, S, D = 16, 2048, 768
H, E = 192, 4
N_CORES = 8
TOK_TOTAL = B * S                 # 32768
TOK = TOK_TOTAL // N_CORES        # 4096 tokens per core
TILE = 512                        # tokens per pipeline tile
N_TILES = TOK // TILE             # 8
SUBT = TILE // 128                # 4 token subtiles of 128
KC = D // 128                     # 6 contraction chunks
DC = KC // 2                      # 3 DoubleRow double-chunks

XS = 8.0                          # x fp8 scale
WS = 64.0                         # weight fp8 scale
OS = 64.0                         # output / skip scale
MBIG = float(2 ** 20)             # pre-gelu mask offset (=> -2048 post scale)

_NC_CACHE = None


def _build_bass():
    dt = mybir.dt
    DR = mybir.MatmulPerfMode.DoubleRow
    nc = bacc.Bacc("TRN2", target_bir_lowering=False)

    xf16 = nc.dram_tensor("xf16", [D, TOK], dt.float16, kind="ExternalInput")
    x8 = nc.dram_tensor("x8", [D, TOK], dt.float8e4, kind="ExternalInput")
    xs16 = nc.dram_tensor("xs16", [TOK, D], dt.float16, kind="ExternalInput")
    w18 = nc.dram_tensor("w18", [D, D], dt.float8e4, kind="ExternalInput")
    w28 = nc.dram_tensor("w28", [D, D], dt.float8e4, kind="ExternalInput")
    rw16 = nc.dram_tensor("rw16", [D, E], dt.float16, kind="ExternalInput")
    rbt = nc.dram_tensor("rbt", [32, E], dt.float32, kind="ExternalInput")
    b1r = nc.dram_tensor("b1r", [128, KC], dt.float32, kind="ExternalInput")
    een = nc.dram_tensor("een", [E, 128], dt.bfloat16, kind="ExternalInput")
    b2s = nc.dram_tensor("b2s", [E, D], dt.bfloat16, kind="ExternalInput")
    out = nc.dram_tensor("out", [TOK, D], dt.float16, kind="ExternalOutput")

    xf_r = xf16.rearrange("(c p) t -> p c t", p=128)
    x8_r = x8.rearrange("(c i p) t -> p c i t", i=2, p=128)
    w1_r = w18.rearrange("(c i p) m -> p c i m", i=2, p=128)
    w2_r = w28.rearrange("(c i p) m -> p c i m", i=2, p=128)
    rw_r = rw16.rearrange("(c p) e -> p c e", p=128)
    xs_r = xs16.rearrange("(n p) d -> p n d", p=128)
    out_r = out.rearrange("(n p) d -> p n d", p=128)

    add = mybir.AluOpType.add
    amax = mybir.AluOpType.max
    iseq = mybir.AluOpType.is_equal
    gelu = mybir.ActivationFunctionType.Gelu

    with tile.TileContext(nc) as tc:
        with (
            tc.tile_pool(name="const", bufs=1) as const,
            tc.tile_pool(name="xin", bufs=3) as xin,
            tc.tile_pool(name="hbuf", bufs=2) as hbuf,
            tc.tile_pool(name="obuf", bufs=6) as obuf,
            tc.tile_pool(name="small", bufs=3) as small,
            tc.tile_pool(name="ps_rt", bufs=2, space="PSUM") as ps_rt,
            tc.tile_pool(name="ps_h", bufs=2, space="PSUM") as ps_h,
            tc.tile_pool(name="ps_y5", bufs=2, space="PSUM") as ps_y5,
            tc.tile_pool(name="ps_y2", bufs=2, space="PSUM") as ps_y2,
        ):
            # small constants ride the gpsimd SWDGE queue; the two weight
            # matrices go on the sync HWDGE FIFO interleaved with tile-0 loads.
            rwsb = const.tile([128, KC, E], dt.float16)
            nc.gpsimd.dma_start(rwsb, rw_r)
            rbsb = const.tile([32, E], dt.float32)
            nc.gpsimd.dma_start(rbsb, rbt[:])
            b1sb = const.tile([128, KC], dt.float32)
            nc.gpsimd.dma_start(b1sb, b1r[:])
            eesb = const.tile([E, 128], dt.bfloat16)
            nc.gpsimd.dma_start(eesb, een[:])
            b2sb = const.tile([E, D], dt.bfloat16)
            nc.gpsimd.dma_start(b2sb, b2s[:])
            w1sb = const.tile([128, DC, 2, D], dt.float8e4)
            w2sb = const.tile([128, DC, 2, D], dt.float8e4)

            def load_tiles(it):
                t0 = it * TILE
                xf = xin.tile([128, KC, TILE], dt.float16, tag="xf")
                nc.sync.dma_start(xf, xf_r[:, :, t0 : t0 + TILE])
                x8t = xin.tile([128, DC, 2, TILE], dt.float8e4, tag="x8t")
                nc.sync.dma_start(x8t, x8_r[:, :, :, t0 : t0 + TILE])
                if it == 0:
                    nc.sync.dma_start(w1sb, w1_r)
                xst = xin.tile([128, SUBT, D], dt.float16, tag="xst")
                nc.scalar.dma_start(xst, xs_r[:, it * SUBT : (it + 1) * SUBT, :])
                if it == 0:
                    nc.sync.dma_start(w2sb, w2_r)
                return xf, x8t, xst

            def router_pe(xf):
                """fp16 logits^T for one tile -> psum [4, TILE]."""
                psrt = ps_rt.tile([E, TILE], dt.float32, tag="psrt")
                for kc in range(KC):
                    nc.tensor.matmul(
                        psrt, rwsb[:, kc, :], xf[:, kc, :],
                        start=(kc == 0), stop=(kc == KC - 1),
                    )
                return psrt

            def router_mask(psrt):
                """one-hot top-1 mask mt [4, TILE] bf16 via 32x32 transposes."""
                G = TILE // 32
                lt32s = small.tile([32, TILE], dt.float32, tag="lt32s")
                nc.scalar.copy(lt32s[0:E], psrt)
                # token-major blocks: lt32[p, 32g+r] = lt32s[r, 32g+p]
                lt32 = small.tile([32, TILE], dt.float32, tag="lt32")
                nc.vector.transpose(lt32, lt32s)
                v = lt32.rearrange("p (g r) -> p g r", r=32)
                lt_tok = small.tile([32, G, E], dt.float32, tag="lt_tok")
                nc.vector.tensor_tensor(
                    lt_tok, v[:, :, 0:E],
                    rbsb[:, None, :].to_broadcast((32, G, E)), add,
                )
                mxg = small.tile([32, G], dt.float32, tag="mxg")
                nc.vector.tensor_reduce(
                    out=mxg, in_=lt_tok, axis=mybir.AxisListType.X, op=amax
                )
                mtb = small.tile([32, TILE], dt.bfloat16, tag="mtb")
                mview = mtb.rearrange("p (g r) -> p g r", r=32)
                nc.vector.tensor_tensor(
                    mview[:, :, 0:E], lt_tok,
                    mxg[:, :, None].to_broadcast((32, G, E)), iseq,
                )
                # back-transpose: mt32[e, t] = one_hot[t, e] for e < 4
                mt32 = small.tile([32, TILE], dt.bfloat16, tag="mt32")
                nc.vector.transpose(mt32, mtb)
                return mt32[0:E]

            # PE warm-up burst: spin the HAM up during the DMA head
            dummy = const.tile([128, TILE], dt.bfloat16)
            nc.vector.memset(dummy, 0.0)
            psd = ps_h.tile([128, TILE], dt.float32, tag="psh")
            for _ in range(10):
                nc.tensor.matmul(psd, dummy[:, 0:128], dummy, start=True, stop=True)

            tiles = {0: load_tiles(0)}
            mt = router_mask(router_pe(tiles[0][0]))
            tiles[1] = load_tiles(1)

            for it in range(N_TILES):
                xf, x8t, xst = tiles[it]
                if it + 2 < N_TILES:
                    tiles[it + 2] = load_tiles(it + 2)

                # ---- mm1 (fp8 DoubleRow) + mask-bias + gelu -> mh fp8 ----
                mh = hbuf.tile([128, KC, TILE], dt.float8e4, tag="mh")
                for hc in range(KC):
                    psh = ps_h.tile([128, TILE], dt.float32, tag="psh")
                    # -2^20 into non-selected experts' rows (gelu() -> 0)
                    nc.tensor.matmul(psh, eesb, mt, start=True, stop=False)
                    for dc in range(DC):
                        nc.tensor.matmul(
                            psh,
                            w1sb[:, dc, :, hc * 128 : (hc + 1) * 128],
                            x8t[:, dc, :, :],
                            start=False, stop=(dc == DC - 1),
                            perf_mode=DR,
                        )
                    nc.scalar.activation(
                        mh[:, hc, :], psh, gelu,
                        bias=b1sb[:, hc : hc + 1], scale=1.0 / (XS * WS),
                    )

                # router for tile n+1 hides its DVE chain under mm2
                psrt_n = router_pe(tiles[it + 1][0]) if it + 1 < N_TILES else None

                # ---- mm2 (fp8 DoubleRow) + b2 + skip-add, token-major ----
                for a in range(SUBT):
                    osb = obuf.tile([128, D], dt.float16, tag="osb")
                    psy5 = ps_y5.tile([128, 512], dt.float32, tag="psy5")
                    psy2 = ps_y2.tile([128, 256], dt.float32, tag="psy2")
                    mta = mt[:, a * 128 : (a + 1) * 128]
                    nc.tensor.matmul(psy5, mta, b2sb[:, 0:512], start=True, stop=False)
                    nc.tensor.matmul(psy2, mta, b2sb[:, 512:768], start=True, stop=False)
                    for dc in range(DC):
                        mha = mh[:, 2 * dc : 2 * dc + 2, a * 128 : (a + 1) * 128]
                        nc.tensor.matmul(
                            psy5, mha, w2sb[:, dc, :, 0:512],
                            start=False, stop=(dc == DC - 1), perf_mode=DR,
                        )
                        nc.tensor.matmul(
                            psy2, mha, w2sb[:, dc, :, 512:768],
                            start=False, stop=(dc == DC - 1), perf_mode=DR,
                        )
                    nc.vector.tensor_tensor(osb[:, 0:512], psy5, xst[:, a, 0:512], add)
                    nc.vector.tensor_tensor(osb[:, 512:768], psy2, xst[:, a, 512:768], add)
                    nc.scalar.dma_start(out_r[:, it * SUBT + a, :], osb)

                if psrt_n is not None:
                    mt = router_mask(psrt_n)
                del tiles[it]

    nc.compile()
    return nc


def _prep_inputs(x, router_w, router_b, w1, b1, w2, b2):
    """Host-side packing: cast/scale/transpose; returns per-core input dicts."""
    xf = np.ascontiguousarray(np.asarray(x, dtype=F32).reshape(TOK_TOTAL, D))

    rw = np.asarray(router_w, dtype=F32).astype(FP16)         # [D, E]
    rb = np.ascontiguousarray(
        np.tile(np.asarray(router_b, dtype=F32).reshape(1, E), (32, 1))
    )

    w1f = np.asarray(w1, dtype=F32)                           # [E, D, H]
    w2f = np.asarray(w2, dtype=F32)                           # [E, H, D]
    b1f = np.asarray(b1, dtype=F32)                           # [E, H]
    b2f = np.asarray(b2, dtype=F32)                           # [E, D]

    # experts interleaved along the stacked hidden dim (unit j of expert e at
    # index 4j + e) so the mask pattern repeats identically per 128-row chunk.
    w1s = np.ascontiguousarray(w1f.transpose(1, 2, 0).reshape(D, H * E))
    w2s = np.ascontiguousarray(w2f.transpose(1, 0, 2).reshape(H * E, D))
    w18 = (WS * w1s).astype(FP8)
    w28 = (WS * w2s).astype(FP8)
    b1all = np.ascontiguousarray(b1f.T.reshape(E * H))
    b1r = np.ascontiguousarray(b1all.reshape(KC, 128).T).astype(F32)
    b2sb = (OS * b2f).astype(BF16)

    een = np.full((E, 128), -MBIG, dtype=BF16)
    for e in range(E):
        een[e, e::E] = 0.0

    in_maps = []
    for c in range(N_CORES):
        sl = slice(c * TOK, (c + 1) * TOK)
        xc = xf[sl]
        xT = np.ascontiguousarray(xc.T)
        in_maps.append(
            {
                "xf16": xT.astype(FP16),
                "x8": (XS * xT).astype(FP8),
                "xs16": (OS * xc).astype(FP16),
                "w18": w18,
                "w28": w28,
                "rw16": rw,
                "rbt": rb,
                "b1r": b1r,
                "een": een,
                "b2s": b2sb,
            }
        )
    return in_maps


def _get_nc():
    global _NC_CACHE
    if _NC_CACHE is None:
        _NC_CACHE = _build_bass()
    return _NC_CACHE


def kernel(x, router_w, router_b, w1, b1, w2, b2, _trace=False, _trace_kwargs=None):
    in_maps = _prep_inputs(x, router_w, router_b, w1, b1, w2, b2)
    nc = _get_nc()
    res = run_bass_kernel_spmd(
        nc,
        in_maps,
        core_ids=list(range(N_CORES)),
        trace=_trace,
        **(_trace_kwargs or {}),
    )
    outs = [np.asarray(r["out"], dtype=F32) * (1.0 / OS) for r in res.results]
    full = np.concatenate(outs, axis=0).reshape(B, S, D)
    if _trace:
        kernel.last_results = res
    return full
